# revision 7
# baseline (speedup 1.0000x reference)
"""Trainium2 Bass kernel for BatchIrregularDownsample2d (D=2).

Contract: kernel(**inputs) takes the FULL inputs
    input:        [B, C, N]  float32
    pooling_mask: [B, 1, H, W] int32
and returns the FULL output [B, C, M] float32, where M is the max
per-batch compacted length (identical across batches for quadtree masks
with equal level histograms, which is what this module produces).

Strategy (pure data-parallel over B, one batch per NeuronCore):
  The reference gather G[b] splits into
    - an identity prefix  out[:, :start]            = in[:, :start]
    - a small gather      out[:, start:start+ng]    = in[:, start + rel[j]]
  where rel[j] < nelems = N - start fits in int16.
  Per core: DRAM->DRAM DMA for the prefix copy; DMA the gather source
  region [C, nelems] into SBUF (two 128-partition chunks), run the
  GPSIMD ap_gather custom op (free-dim gather, 16-partition-wrapped
  int16 indices, replicated per Q7 core), DMA the compacted tokens back
  out. Index arithmetic is host-side numpy (as in the original module,
  which syncs the mask to host anyway).
"""

import numpy as np

from concourse import bass, library_config, mybir
from concourse.bass_utils import run_bass_kernel_spmd

f32 = mybir.dt.float32
i16 = mybir.dt.int16

_NUM_CORES = 8


# ---------------------------------------------------------------------------
# Host-side index computation (replicates reference._build_indices, D=2)
# ---------------------------------------------------------------------------

def _batch_indices(mask2d):
    """mask2d: [H, W] int32 quadtree mask. Returns (start, rel_idx int32[ng])
    with absolute gather index = start + rel_idx."""
    D = 2
    s = 2 ** (D - 1)
    start = 0
    for i in range(D - 1):
        start += int((mask2d == i).sum()) // (4 ** i)
    cs = (mask2d >= D - 1)[::s, ::s]
    dt = (mask2d < D)[::s, ::s]
    r, c = np.nonzero(cs)
    topleft = ((r % 2) + (c % 2)) == 0
    dt_at = dt[r, c]
    keep_lower = topleft & ~dt_at
    pos = np.arange(r.shape[0])
    rel = np.concatenate([pos[dt_at], pos[keep_lower]]).astype(np.int64)
    return start, rel, int(r.shape[0])


def _wrap_idxs(rel, num_idxs_pad):
    """Pack indices into the ap_gather layout: int16 [128, num_idxs_pad//16],
    index j at partition j%16, slot j//16, replicated across 8 Q7 groups."""
    padded = np.zeros(num_idxs_pad, np.int16)
    padded[: len(rel)] = rel
    wrapped = padded.reshape(num_idxs_pad // 16, 16).T  # [16, S]
    return np.tile(wrapped, (8, 1)).copy()  # [128, S]


# ---------------------------------------------------------------------------
# Bass program
# ---------------------------------------------------------------------------

_prog_cache = {}


def _build_program(C, N, start, ng, M, n_iters):
    """One batch per core: input [C, N] -> output [C, M]."""
    key = (C, N, start, ng, M, n_iters)
    if key in _prog_cache:
        return _prog_cache[key]

    assert C == 256, "kernel assumes two 128-partition C chunks"
    nelems = N - start                       # gather source region length
    num_idxs = ((ng + 31) // 32) * 32        # pad to %32 for ap_gather
    S = num_idxs // 16
    assert nelems <= 2 ** 15, nelems         # int16 per-partition addressing
    assert num_idxs % 16 == 0

    nc = bass.Bass("TRN2")
    inp = nc.dram_tensor("input", [C, N], f32, kind="ExternalInput").ap()
    idxs = nc.dram_tensor("idxs", [128, S], i16, kind="ExternalInput").ap()
    out = nc.dram_tensor("output", [C, M], f32, kind="ExternalOutput").ap()

    src0 = nc.alloc_sbuf_tensor("src0", [128, nelems], f32).ap()
    src1 = nc.alloc_sbuf_tensor("src1", [128, nelems], f32).ap()
    og0 = nc.alloc_sbuf_tensor("og0", [128, num_idxs], f32).ap()
    og1 = nc.alloc_sbuf_tensor("og1", [128, num_idxs], f32).ap()
    idxt = nc.alloc_sbuf_tensor("idxt", [128, S], i16).ap()

    K = n_iters
    with (
        nc.Block() as block,
        nc.semaphore("sL0") as sL0,   # src0 region load       (+16 each)
        nc.semaphore("sL1") as sL1,   # src1 region load       (+16 each)
        nc.semaphore("sC") as sC,     # prefix copies          (+16 each)
        nc.semaphore("sI") as sI,     # idx load               (+16)
        nc.semaphore("sS") as sS,     # gather-out stores      (+16 each)
        nc.semaphore("gp") as gp,     # ap_gather completions  (+1 each)
    ):

        @block.sync
        def _(sync):
            for k in range(K):
                if k > 0:
                    # src tiles are reused; previous gathers must be done
                    sync.wait_ge(gp, 2 * k)
                sync.dma_start(out=src0[:], in_=inp[0:128, start:N]).then_inc(sL0, 16)
                sync.dma_start(out=src1[:], in_=inp[128:256, start:N]).then_inc(sL1, 16)
                sync.dma_start(
                    out=out[0:128, 0:start], in_=inp[0:128, 0:start]
                ).then_inc(sC, 16)
                sync.dma_start(
                    out=out[128:256, 0:start], in_=inp[128:256, 0:start]
                ).then_inc(sC, 16)
            sync.wait_ge(sC, 32 * K)

        @block.scalar
        def _(scalar):
            scalar.dma_start(out=idxt[:], in_=idxs[:]).then_inc(sI, 16)
            for k in range(K):
                scalar.wait_ge(gp, 2 * k + 1)
                scalar.dma_start(
                    out=out[0:128, start : start + ng], in_=og0[:, 0:ng]
                ).then_inc(sS, 16)
                scalar.wait_ge(gp, 2 * k + 2)
                scalar.dma_start(
                    out=out[128:256, start : start + ng], in_=og1[:, 0:ng]
                ).then_inc(sS, 16)
            scalar.wait_ge(sS, 32 * K)

        @block.gpsimd
        def _(g):
            g.load_library(library_config.ap_gather)
            g.wait_ge(sI, 16)
            for k in range(K):
                if k > 0:
                    # og tiles are reused; previous stores must be done
                    g.wait_ge(sS, 32 * k)
                g.wait_ge(sL0, 16 * (k + 1))
                g.ap_gather(
                    out_ap=og0[:],
                    in_ap=src0[:],
                    idxs_ap=idxt[:],
                    channels=128,
                    num_elems=nelems,
                    d=1,
                    num_idxs=num_idxs,
                ).then_inc(gp, 1)
                g.wait_ge(sL1, 16 * (k + 1))
                g.ap_gather(
                    out_ap=og1[:],
                    in_ap=src1[:],
                    idxs_ap=idxt[:],
                    channels=128,
                    num_elems=nelems,
                    d=1,
                    num_idxs=num_idxs,
                ).then_inc(gp, 1)

    # Populate .instr bytes for extended-inst InstISA subclasses (APGather,
    # PseudoReloadLibraryIndex). Raw Bass doesn't run this pass; without it
    # walrus fails with "ISA wrong length".
    mybir.codegen_inst_isa_subclasses(nc)

    _prog_cache[key] = (nc, num_idxs)
    return nc, num_idxs


# ---------------------------------------------------------------------------
# Public entry point
# ---------------------------------------------------------------------------

def kernel(input, pooling_mask, _n_iters=1):
    x = np.asarray(input)
    mask = np.asarray(pooling_mask)
    B, C, N = x.shape
    assert x.dtype == np.float32

    per_batch = [_batch_indices(mask[b, 0]) for b in range(B)]
    starts = {s for s, _, _ in per_batch}
    ngs = {len(r) for _, r, _ in per_batch}
    M = max(s + len(r) for s, r, _ in per_batch)

    if len(starts) != 1 or len(ngs) != 1 or B != _NUM_CORES or C != 256:
        # Irregular shape structure across batches (not produced by this
        # module's mask builder) — fall back to a host gather.
        out = np.zeros((B, C, M), np.float32)
        for b, (s, rel, _) in enumerate(per_batch):
            n = s + len(rel)
            g = np.concatenate([np.arange(s, dtype=np.int64), s + rel])
            out[b, :, :n] = x[b][:, g]
        return out

    start = per_batch[0][0]
    ng = len(per_batch[0][1])

    nc, num_idxs = _build_program(C, N, start, ng, M, _n_iters)
    in_maps = [
        {
            "input": np.ascontiguousarray(x[b]),
            "idxs": _wrap_idxs(per_batch[b][1], num_idxs),
        }
        for b in range(B)
    ]
    res = run_bass_kernel_spmd(nc, in_maps, list(range(_NUM_CORES)))
    return np.stack([res.results[b]["output"] for b in range(B)])


# revision 20
# speedup vs baseline: 1.5756x; 1.5756x over previous
"""Trainium2 Bass kernel for BatchIrregularDownsample2d (D=2).

Contract: kernel(**inputs) takes the FULL inputs
    input:        [B, C, N]  float32
    pooling_mask: [B, 1, H, W] int32
and returns the FULL output [B, C, M] float32, where M is the max
per-batch compacted length (identical across batches for quadtree masks
with equal level histograms, which is what this module produces).

Strategy (pure data-parallel over B, one batch per NeuronCore):
  The reference gather G[b] splits into
    - an identity prefix  out[:, :start]            = in[:, :start]
    - a small gather      out[:, start:start+ng]    = in[:, start + rel[j]]
  where rel[j] < nelems = N - start fits in int16.
  Per core: DRAM->DRAM DMA for the prefix copy. The gather source region
  [C=256, nelems] is loaded in stages, interleaved by the DVE into one
  SBUF buffer srcI[128, nelems, 2] holding both 128-partition C-chunks
  elementwise-interleaved, then a single GPSIMD ap_gather with d=2
  gathers both chunks per index (the op's cost is per 4-index request,
  so d=2 halves it vs. two d=1 calls). Stores DMA the two interleaved
  planes of the result straight out with strided reads.
  Index arithmetic is host-side numpy (as in the original torch module,
  which syncs the mask to host anyway).
"""

import numpy as np

from concourse import bass, library_config, mybir
from concourse.bass_utils import run_bass_kernel_spmd

f32 = mybir.dt.float32
i16 = mybir.dt.int16

_NUM_CORES = 8


# ---------------------------------------------------------------------------
# Host-side index computation (replicates reference._build_indices, D=2)
# ---------------------------------------------------------------------------

def _batch_indices(mask2d):
    """mask2d: [H, W] int32 quadtree mask. Returns (start, rel_idx int64[ng])
    with absolute gather index = start + rel_idx."""
    D = 2
    s = 2 ** (D - 1)
    start = 0
    for i in range(D - 1):
        start += int((mask2d == i).sum()) // (4 ** i)
    cs = (mask2d >= D - 1)[::s, ::s]
    dt = (mask2d < D)[::s, ::s]
    r, c = np.nonzero(cs)
    topleft = ((r % 2) + (c % 2)) == 0
    dt_at = dt[r, c]
    keep_lower = topleft & ~dt_at
    pos = np.arange(r.shape[0])
    rel = np.concatenate([pos[dt_at], pos[keep_lower]]).astype(np.int64)
    return start, rel, int(r.shape[0])


def _wrap_idxs(rel, num_idxs_pad):
    """Pack indices into the ap_gather layout: int16 [128, num_idxs_pad//16],
    index j at partition j%16, slot j//16, replicated across 8 Q7 groups."""
    padded = np.zeros(num_idxs_pad, np.int16)
    padded[: len(rel)] = rel
    wrapped = padded.reshape(num_idxs_pad // 16, 16).T  # [16, S]
    return np.tile(wrapped, (8, 1)).copy()  # [128, S]


# ---------------------------------------------------------------------------
# Bass program
# ---------------------------------------------------------------------------

_prog_cache = {}

_N_SUB = 8  # gather-region load sub-chunks (2 alternating stage slots / chunk)


def _build_program(C, N, start, ng, M, n_iters, parts=("copy", "load", "gather", "store")):
    """One batch per core: input [C, N] -> output [C, M].

    `parts` selects pipeline stages (for component benchmarking): any
    subset of {copy, load, gather, store}; gather needs load, store needs
    gather."""
    key = (C, N, start, ng, M, n_iters, tuple(parts))
    if key in _prog_cache:
        return _prog_cache[key]
    do_copy = "copy" in parts
    do_load = "load" in parts
    do_gather = "gather" in parts and do_load
    do_store = "store" in parts and do_gather

    assert C == 256, "kernel assumes two 128-partition C chunks"
    nelems = N - start                       # gather source region length
    num_idxs = ((ng + 31) // 32) * 32        # pad to %32 for ap_gather
    S = num_idxs // 16
    assert nelems * 2 <= 2 ** 15, nelems     # int16 cell addressing, d=2
    E = (nelems + _N_SUB - 1) // _N_SUB      # stage size
    subs = [(e * E, min(nelems, (e + 1) * E)) for e in range(_N_SUB)]

    nc = bass.Bass("TRN2")
    inp = nc.dram_tensor("input", [C, N], f32, kind="ExternalInput").ap()
    idxs = nc.dram_tensor("idxs", [128, S], i16, kind="ExternalInput").ap()
    out = nc.dram_tensor("output", [C, M], f32, kind="ExternalOutput").ap()

    # Alternating stage slots per C-chunk (a: chunk c0..127, b: c128..255)
    stga = [nc.alloc_sbuf_tensor(f"stga{i}", [128, E], f32).ap() for i in range(2)]
    stgb = [nc.alloc_sbuf_tensor(f"stgb{i}", [128, E], f32).ap() for i in range(2)]
    srcI = nc.alloc_sbuf_tensor("srcI", [128, nelems, 2], f32).ap()
    ogI = nc.alloc_sbuf_tensor("ogI", [128, num_idxs, 2], f32).ap()
    ogDe = nc.alloc_sbuf_tensor("ogDe", [128, num_idxs], f32).ap()
    idxt = nc.alloc_sbuf_tensor("idxt", [128, S], i16).ap()

    K = n_iters
    with (
        nc.Block() as block,
        nc.semaphore("se0") as se0,   # even sub-chunk loads (2x16 per sub)
        nc.semaphore("se1") as se1,   # odd  sub-chunk loads
        nc.semaphore("sC") as sC,     # prefix copies          (+16 each)
        nc.semaphore("sI") as sI,     # idx load               (+16)
        nc.semaphore("sS0") as sS0,   # plane-0 stores         (+16 each)
        nc.semaphore("sS1") as sS1,   # plane-1 stores         (+16 each)
        nc.semaphore("vI") as vI,     # DVE interleave copies  (+1 each)
        nc.semaphore("vD") as vD,     # DVE de-interleave copies (+1 each)
        nc.semaphore("gp") as gp,     # ap_gather completions  (+1 each)
    ):
        sub_sems = [se0, se1]

        @block.sync
        def _(sync):
            for k in range(K):
                if do_load:
                    for e, (lo, hi) in enumerate(subs):
                        if do_gather:
                            # stage slot reused from sub-chunk e-2: its two
                            # interleave copies must be done
                            sync.wait_ge(vI, max(0, 16 * k + 2 * (e - 1)))
                            # self-wait on the slot sem so its next updates
                            # are provably ordered (race-detector hygiene;
                            # implied by the vI wait above)
                            sync.wait_ge(
                                sub_sems[e % 2], 32 * (k * (_N_SUB // 2) + e // 2)
                            )
                        sync.dma_start(
                            out=stga[e % 2][:, 0 : hi - lo],
                            in_=inp[0:128, start + lo : start + hi],
                        ).then_inc(sub_sems[e % 2], 16)
                        sync.dma_start(
                            out=stgb[e % 2][:, 0 : hi - lo],
                            in_=inp[128:256, start + lo : start + hi],
                        ).then_inc(sub_sems[e % 2], 16)
                if do_copy:
                    sync.dma_start(
                        out=out[0:128, 0:start], in_=inp[0:128, 0:start]
                    ).then_inc(sC, 16)
                    sync.dma_start(
                        out=out[128:256, 0:start], in_=inp[128:256, 0:start]
                    ).then_inc(sC, 16)
            if do_copy:
                sync.wait_ge(sC, 32 * K)
            if do_load and not do_gather:
                sync.wait_ge(se0, 16 * K * _N_SUB)
                sync.wait_ge(se1, 16 * K * _N_SUB)

        @block.vector
        def _(vec):
            if not do_gather:
                return

            def deinterleave(k):
                # ogI(k) -> ogDe plane copies; p1 waits for p0's store so
                # ogDe can be reused. Runs before iter k+1's interleaves so
                # ogI frees up for gather k+1.
                vec.wait_ge(gp, k + 1)
                if do_store and k > 0:
                    vec.wait_ge(sS1, 16 * k)  # ogDe last read by store1(k-1)
                vec.tensor_copy(ogDe[:], ogI[:, :, 0]).then_inc(vD, 1)
                if do_store:
                    vec.wait_ge(sS0, 16 * (k + 1))
                vec.tensor_copy(ogDe[:], ogI[:, :, 1]).then_inc(vD, 1)

            for k in range(K):
                if k > 0:
                    deinterleave(k - 1)
                for e, (lo, hi) in enumerate(subs):
                    # both loads of this sub-chunk slot done (cumulative:
                    # slot e%2 sees 32 increments per use)
                    n_uses = k * (_N_SUB // 2) + e // 2 + 1
                    vec.wait_ge(sub_sems[e % 2], 32 * n_uses)
                    if e == 0:
                        # srcI overwrite: previous gather must be done
                        vec.wait_ge(gp, k)
                    vec.tensor_copy(
                        srcI[:, lo:hi, 0], stga[e % 2][:, 0 : hi - lo]
                    ).then_inc(vI, 1)
                    vec.tensor_copy(
                        srcI[:, lo:hi, 1], stgb[e % 2][:, 0 : hi - lo]
                    ).then_inc(vI, 1)
            deinterleave(K - 1)

        @block.scalar
        def _(scalar):
            if do_gather:
                scalar.dma_start(out=idxt[:], in_=idxs[:]).then_inc(sI, 16)
            if do_store:
                for k in range(K):
                    scalar.wait_ge(vD, 2 * k + 1)
                    scalar.dma_start(
                        out=out[0:128, start : start + ng], in_=ogDe[:, 0:ng]
                    ).then_inc(sS0, 16)
                    scalar.wait_ge(vD, 2 * k + 2)
                    scalar.dma_start(
                        out=out[128:256, start : start + ng], in_=ogDe[:, 0:ng]
                    ).then_inc(sS1, 16)
                scalar.wait_ge(sS0, 16 * K)
                scalar.wait_ge(sS1, 16 * K)

        @block.gpsimd
        def _(g):
            if not do_gather:
                return
            g.load_library(library_config.ap_gather)
            g.wait_ge(sI, 16)
            for k in range(K):
                g.wait_ge(vI, 16 * (k + 1))
                if k > 0:
                    g.wait_ge(vD, 2 * k)  # ogI reused; prev de-interleave done
                g.ap_gather(
                    out_ap=ogI[:],
                    in_ap=srcI[:],
                    idxs_ap=idxt[:],
                    channels=128,
                    num_elems=nelems,
                    d=2,
                    num_idxs=num_idxs,
                ).then_inc(gp, 1)

    # Populate .instr bytes for extended-inst InstISA subclasses (APGather,
    # PseudoReloadLibraryIndex). Raw Bass doesn't run this pass; without it
    # walrus fails with "ISA wrong length".
    mybir.codegen_inst_isa_subclasses(nc)

    _prog_cache[key] = (nc, num_idxs)
    return nc, num_idxs


# ---------------------------------------------------------------------------
# Public entry point
# ---------------------------------------------------------------------------

def kernel(input, pooling_mask, _n_iters=1):
    x = np.asarray(input)
    mask = np.asarray(pooling_mask)
    B, C, N = x.shape
    assert x.dtype == np.float32

    per_batch = [_batch_indices(mask[b, 0]) for b in range(B)]
    starts = {s for s, _, _ in per_batch}
    ngs = {len(r) for _, r, _ in per_batch}
    M = max(s + len(r) for s, r, _ in per_batch)

    if len(starts) != 1 or len(ngs) != 1 or B != _NUM_CORES or C != 256:
        # Irregular shape structure across batches (not produced by this
        # module's mask builder) — fall back to a host gather.
        out = np.zeros((B, C, M), np.float32)
        for b, (s, rel, _) in enumerate(per_batch):
            n = s + len(rel)
            g = np.concatenate([np.arange(s, dtype=np.int64), s + rel])
            out[b, :, :n] = x[b][:, g]
        return out

    start = per_batch[0][0]
    ng = len(per_batch[0][1])

    nc, num_idxs = _build_program(C, N, start, ng, M, _n_iters)
    in_maps = [
        {
            "input": np.ascontiguousarray(x[b]),
            "idxs": _wrap_idxs(per_batch[b][1], num_idxs),
        }
        for b in range(B)
    ]
    res = run_bass_kernel_spmd(nc, in_maps, list(range(_NUM_CORES)))
    return np.stack([res.results[b]["output"] for b in range(B)])
